# revision 25
# baseline (speedup 1.0000x reference)
"""Trainium2 Bass kernel for the 2-module Mamba-style SSM block.

Sharding: 8 cores = 4 batches x 2 modules (core c -> batch c//2, module c%2).
Each core computes one full branch for one batch; aggregate+out_proj folded
into M_k per module; pair-wise AllReduce; host picks one core per batch.

v2: channels on partitions, fp16 cube, L chunked at LC=512.
Per chunk: in_proj (PE f16) with conv + x_proj interleaved per channel
tile; dt proj + softplus as two batched ACT passes (avoids per-tile
Exp<->Ln table thrash); cube per channel tile:
  a = exp(A_n * delta) fused on ACT (per-partition scale), u = v (x) B
  via one broadcast-AP f16 tensor_tensor, the 16 state recurrences run
  as TWO 8-state tensor_tensor_scans (state boundaries reset by zeroing
  a[:, n, 0]; chunk carries folded into u[:, :, 0]), scan output lands
  in-place over u, then C-mult + log2 tree-reduce, gating (all DVE
  fp16 2x mode; big elementwise ops are kept OFF GpSimd because
  concurrent Pool ops starve DVE SBUF access), out_proj (PE, f16 yf
  stationary blocks, streamed Mk) accumulated over channel tiles.
GpSimd handles only tiny strided ops (halo moves, carry extract/fold).
One pair AllReduce at the end (chunked collectives contend with
compute globally and slow every engine ~20%).
"""
from contextlib import ExitStack

import numpy as np

import concourse.bass as bass
import concourse.tile as tile
from concourse import bacc, mybir
from concourse.bass_utils import run_bass_kernel_spmd

FP = mybir.dt.float32
F16 = mybir.dt.float16
AX = mybir.AxisListType
OP = mybir.AluOpType
AF = mybir.ActivationFunctionType

B, L, D = 4, 2048, 1024
E, N, CW, K, R = 2048, 16, 4, 2, 64
ET = E // 128           # 16 channel tiles
DT = D // 128           # 8 d_model tiles
LC = 512                # chunk length along L
NLC = L // LC           # 4 chunks
MMF = 512               # matmul moving free size
NCORES = 8

_CACHE = {}


def _build_program():
    nc = bacc.Bacc("TRN2", target_bir_lowering=False, debug=False,
                   num_devices=NCORES)

    def din(name, shape, dt=F16):
        return nc.dram_tensor(name, list(shape), dt, kind="ExternalInput").ap()

    hsT = din("hsT", (D, L))              # hidden_states[b].T, f16
    winT = din("winT", (D, 2 * E))        # in_proj_w.T, f16
    xpT = din("xpT", (E, R + 2 * N))      # x_proj_w.T, f16
    dtwT = din("dtwT", (R, E))            # dt_w[k].T, f16
    dtb = din("dtb", (E, 1), FP)
    convw = din("convw", (E, CW), FP)
    convb = din("convb", (E, 1), FP)
    Amat = din("Amat", (E, N), FP)        # -exp(A_log[k])
    Dp = din("Dp", (E, 1), FP)
    Mk = din("Mk", (E, D))                # (out_w @ agg_w[:, k*E:(k+1)*E]).T, f16
    out = nc.dram_tensor("out", [L, D], FP, kind="ExternalOutput").ap()

    zspill = nc.dram_tensor("zspill", [E, L], F16).ap()
    bcspill = nc.dram_tensor("bcspill", [2 * N, L], F16).ap()

    with tile.TileContext(nc) as tc, ExitStack() as ctx:
        const = ctx.enter_context(tc.tile_pool(name="const", bufs=1))
        dram = ctx.enter_context(tc.tile_pool(name="dram", bufs=1, space="DRAM"))
        wpool = ctx.enter_context(tc.tile_pool(name="wpool", bufs=2))
        ch_pool = ctx.enter_context(tc.tile_pool(name="chp", bufs=2))
        hs_pool = ctx.enter_context(tc.tile_pool(name="hsp", bufs=1))
        u_pool = ctx.enter_context(tc.tile_pool(name="up", bufs=2))
        a_pool = ctx.enter_context(tc.tile_pool(name="ap", bufs=2))
        dl_pool = ctx.enter_context(tc.tile_pool(name="dlp", bufs=1))
        a0_pool = ctx.enter_context(tc.tile_pool(name="a0p", bufs=1))
        t_pool = ctx.enter_context(tc.tile_pool(name="tp", bufs=1))
        zs_pool = ctx.enter_context(tc.tile_pool(name="zsp", bufs=2))
        y_pool = ctx.enter_context(tc.tile_pool(name="yp", bufs=1))
        ev_pool = ctx.enter_context(tc.tile_pool(name="ev", bufs=1))
        xd_pool = ctx.enter_context(tc.tile_pool(name="xd", bufs=1))
        mk_pool = ctx.enter_context(tc.tile_pool(name="mkp", bufs=1))
        pin = ctx.enter_context(tc.tile_pool(name="pin", bufs=2, space="PSUM"))
        pxp = ctx.enter_context(tc.tile_pool(name="pxp", bufs=2, space="PSUM"))
        pdt = ctx.enter_context(tc.tile_pool(name="pdt", bufs=2, space="PSUM"))
        pout = ctx.enter_context(tc.tile_pool(name="pout", bufs=2, space="PSUM"))

        opart = dram.tile([L, D], FP)
        oshared = dram.tile([L, D], FP)

        # ---- resident constants ----
        xpT_sb = const.tile([128, ET, R + 2 * N], F16)
        nc.sync.dma_start(out=xpT_sb,
                          in_=xpT.rearrange("(a p) c -> p a c", p=128))
        dtwT_sb = const.tile([R, ET, 128], F16)
        nc.sync.dma_start(out=dtwT_sb,
                          in_=dtwT.rearrange("p (a c) -> p a c", c=128))
        Amat_sb = const.tile([128, ET, N], FP)
        nc.sync.dma_start(out=Amat_sb,
                          in_=Amat.rearrange("(a p) n -> p a n", p=128))
        dtb_sb = const.tile([128, ET, 1], FP)
        nc.sync.dma_start(out=dtb_sb, in_=dtb.rearrange("(a p) o -> p a o", p=128))
        Dp_sb = const.tile([128, ET, 1], FP)
        nc.sync.dma_start(out=Dp_sb, in_=Dp.rearrange("(a p) o -> p a o", p=128))
        convw_sb = const.tile([128, ET, CW], FP)
        nc.sync.dma_start(out=convw_sb,
                          in_=convw.rearrange("(a p) c -> p a c", p=128))
        convb_sb = const.tile([128, ET, 1], FP)
        nc.sync.dma_start(out=convb_sb,
                          in_=convb.rearrange("(a p) o -> p a o", p=128))
        carry = const.tile([128, ET, N], FP)
        halo = const.tile([128, ET, CW - 1], F16)
        nc.vector.memset(halo, 0.0)

        for lc in range(NLC):
            lsl = slice(lc * LC, (lc + 1) * LC)
            # ---- in_proj (PE): xz[:, lsl] for all 2E channels ----
            hs_sb = hs_pool.tile([128, DT, LC], F16, tag="hs")
            for dt_ in range(DT):
                nc.sync.dma_start(out=hs_sb[:, dt_, :],
                                  in_=hsT[dt_ * 128:(dt_ + 1) * 128, lsl])
            xtp = ch_pool.tile([128, ET, CW - 1 + LC], F16, tag="xtp")
            psx = pxp.tile([R + 2 * N, LC], FP, tag="mmxp")
            for ct in range(2 * ET):
                win_ct = wpool.tile([128, DT, 128], F16, tag="win")
                nc.sync.dma_start(
                    out=win_ct,
                    in_=winT[:, ct * 128:(ct + 1) * 128].rearrange(
                        "(a p) c -> p a c", p=128))
                psums = []
                for _h in range(LC // MMF):
                    ps_in = pin.tile([128, MMF], FP, tag="mmin", name=f"psin{_h}")
                    psums.append(ps_in)
                for dt_ in range(DT):
                    for h, ps in enumerate(psums):
                        nc.tensor.matmul(ps, win_ct[:, dt_, :],
                                         hs_sb[:, dt_, h * MMF:(h + 1) * MMF],
                                         start=(dt_ == 0), stop=(dt_ == DT - 1))
                if ct < ET:
                    et = ct
                    for h, ps in enumerate(psums):
                        nc.scalar.activation(
                            out=xtp[:, et, CW - 1 + h * MMF:CW - 1 + (h + 1) * MMF],
                            in_=ps, func=AF.Copy)
                    # conv + silu for this tile, then x_proj accumulation
                    nc.gpsimd.tensor_copy(out=xtp[:, et, 0:CW - 1],
                                          in_=halo[:, et, :])
                    acc = t_pool.tile([128, LC], F16, tag="conv")
                    nc.vector.tensor_scalar(out=acc, in0=xtp[:, et, 0:LC],
                                            scalar1=convw_sb[:, et, 0:1],
                                            scalar2=None, op0=OP.mult)
                    for j in range(1, CW):
                        nc.vector.scalar_tensor_tensor(
                            out=acc, in0=xtp[:, et, j:j + LC],
                            scalar=convw_sb[:, et, j:j + 1], in1=acc,
                            op0=OP.mult, op1=OP.add)
                    nc.gpsimd.tensor_copy(out=halo[:, et, :],
                                          in_=xtp[:, et, LC:LC + CW - 1])
                    nc.scalar.activation(out=xtp[:, et, CW - 1:CW - 1 + LC],
                                         in_=acc, func=AF.Silu,
                                         bias=convb_sb[:, et, :], scale=1.0)
                    nc.tensor.matmul(psx, xpT_sb[:, et, :],
                                     xtp[:, et, CW - 1:CW - 1 + LC],
                                     start=(et == 0), stop=(et == ET - 1))
                else:
                    for h, ps in enumerate(psums):
                        zt = ev_pool.tile([128, MMF], F16, tag="zt")
                        nc.scalar.activation(out=zt, in_=ps, func=AF.Silu)
                        nc.sync.dma_start(
                            out=zspill[(ct - ET) * 128:(ct - ET + 1) * 128,
                                       lc * LC + h * MMF:lc * LC + (h + 1) * MMF],
                            in_=zt)

            xdbl = xd_pool.tile([R + 2 * N, LC], F16, tag="xdbl")
            nc.scalar.activation(out=xdbl, in_=psx, func=AF.Copy)
            nc.sync.dma_start(out=bcspill[:, lsl], in_=xdbl[R:R + 2 * N, :])
            Bbc = ch_pool.tile([128, N, LC], F16, tag="Bbc")
            Cbc = ch_pool.tile([128, N, LC], F16, tag="Cbc")
            nc.sync.dma_start(out=Bbc, in_=bass.AP(
                tensor=bcspill.tensor, offset=lc * LC,
                ap=[[0, 128], [L, N], [1, LC]]))
            nc.sync.dma_start(out=Cbc, in_=bass.AP(
                tensor=bcspill.tensor, offset=N * L + lc * LC,
                ap=[[0, 128], [L, N], [1, LC]]))

            # ---- dt proj + softplus in two batched ACT passes ----
            dlt = dl_pool.tile([128, ET, LC], F16, tag="dlt")
            for et in range(ET):
                psd = pdt.tile([128, LC], FP, tag="mmdt")
                nc.tensor.matmul(psd, dtwT_sb[:, et, :], xdbl[0:R, :],
                                 start=True, stop=True)
                nc.scalar.activation(out=dlt[:, et, :], in_=psd, func=AF.Exp,
                                     bias=dtb_sb[:, et, :], scale=1.0)
            for et in range(ET):
                nc.scalar.activation(out=dlt[:, et, :], in_=dlt[:, et, :],
                                     func=AF.Ln, bias=1.0)

            # ---- cube per channel tile ----
            for et in range(ET):
                delta = dlt[:, et, :]
                v = t_pool.tile([128, LC], F16, tag="v")
                nc.vector.tensor_tensor(out=v, in0=delta,
                                        in1=xtp[:, et, CW - 1:CW - 1 + LC],
                                        op=OP.mult)
                vb = v[:, :].rearrange("p (o t) -> p o t", o=1)
                u = u_pool.tile([128, N, LC], F16, tag="u")
                nc.vector.tensor_tensor(out=u,
                                        in0=vb.broadcast_to([128, N, LC]),
                                        in1=Bbc, op=OP.mult)
                if lc > 0:
                    # fold chunk carry into u[:, :, 0]: u0' = u0 + a0 * carry
                    d0f = a0_pool.tile([128, 1], FP, tag="d0f")
                    nc.scalar.activation(out=d0f, in_=delta[:, 0:1], func=AF.Copy)
                    a0 = a0_pool.tile([128, N], F16, tag="a0")
                    nc.scalar.activation(out=a0, in_=Amat_sb[:, et, :],
                                         func=AF.Exp, scale=d0f[:, 0:1])
                    ctmp = a0_pool.tile([128, N], F16, tag="ctmp")
                    nc.vector.tensor_tensor(out=ctmp, in0=a0,
                                            in1=carry[:, et, :], op=OP.mult)
                    ctmp3 = ctmp[:, :].rearrange("p (n o) -> p n o", o=1)
                    nc.gpsimd.tensor_tensor(out=u[:, :, 0:1], in0=u[:, :, 0:1],
                                            in1=ctmp3, op=OP.add)
                for half in range(2):
                    ah = a_pool.tile([128, N // 2, LC], F16, tag="ah",
                                     name=f"ah{half}")
                    for j in range(N // 2):
                        n = half * (N // 2) + j
                        nc.scalar.activation(out=ah[:, j, :], in_=delta,
                                             func=AF.Exp,
                                             scale=Amat_sb[:, et, n:n + 1])
                    # reset recurrence at each state boundary (incl. chunk start)
                    nc.gpsimd.memset(ah[:, :, 0:1], 0.0)
                    uh = u[:, half * (N // 2):(half + 1) * (N // 2), :]
                    nc.vector.tensor_tensor_scan(
                        out=uh.rearrange("p n t -> p (n t)"),
                        data0=ah[:, :, :].rearrange("p n t -> p (n t)"),
                        data1=uh.rearrange("p n t -> p (n t)"),
                        initial=0.0, op0=OP.mult, op1=OP.add)
                if lc < NLC - 1:
                    nc.gpsimd.tensor_copy(out=carry[:, et, :],
                                          in_=u[:, :, LC - 1])
                # C-mult + tree reduce over n (in place on u)
                nc.vector.tensor_tensor(out=u[:, :, :], in0=u[:, :, :],
                                        in1=Cbc, op=OP.mult)
                nc.vector.tensor_tensor(out=u[:, 0:8, :], in0=u[:, 0:8, :],
                                        in1=u[:, 8:16, :], op=OP.add)
                nc.vector.tensor_tensor(out=u[:, 0:4, :], in0=u[:, 0:4, :],
                                        in1=u[:, 4:8, :], op=OP.add)
                nc.vector.tensor_tensor(out=u[:, 0:2, :], in0=u[:, 0:2, :],
                                        in1=u[:, 2:4, :], op=OP.add)
                y = y_pool.tile([128, LC], F16, tag="y")
                nc.vector.tensor_tensor(out=y, in0=u[:, 0, :], in1=u[:, 1, :],
                                        op=OP.add)
                zs = zs_pool.tile([128, LC], F16, tag="zs")
                nc.sync.dma_start(out=zs,
                                  in_=zspill[et * 128:(et + 1) * 128, lsl])
                t2 = t_pool.tile([128, LC], F16, tag="t2")
                nc.vector.scalar_tensor_tensor(out=t2,
                                               in0=xtp[:, et, CW - 1:CW - 1 + LC],
                                               scalar=Dp_sb[:, et, :], in1=y,
                                               op0=OP.mult, op1=OP.add)
                nc.vector.tensor_tensor(out=xtp[:, et, CW - 1:CW - 1 + LC],
                                        in0=t2, in1=zs, op=OP.mult)

            # ---- out_proj (PE): yf^T @ Mk, accumulated over et ----
            for dh in range(D // MMF):
                mk_sb = mk_pool.tile([128, ET, MMF], F16, tag="mk")
                nc.sync.dma_start(
                    out=mk_sb,
                    in_=Mk[:, dh * MMF:(dh + 1) * MMF].rearrange(
                        "(a p) c -> p a c", p=128))
                for tau in range(LC // 128):
                    po = pout.tile([128, MMF], FP, tag="mmo")
                    for et in range(ET):
                        nc.tensor.matmul(
                            po, xtp[:, et, CW - 1 + tau * 128:CW - 1 + (tau + 1) * 128],
                            mk_sb[:, et, :],
                            start=(et == 0), stop=(et == ET - 1))
                    osb = ev_pool.tile([128, MMF], FP, tag="osb")
                    nc.scalar.activation(out=osb, in_=po, func=AF.Copy)
                    nc.sync.dma_start(
                        out=opart[lc * LC + tau * 128:lc * LC + (tau + 1) * 128,
                                  dh * MMF:(dh + 1) * MMF],
                        in_=osb)

        # ---- pair AllReduce + output ----
        nc.gpsimd.collective_compute(
            "AllReduce", OP.add,
            replica_groups=[[0, 1], [2, 3], [4, 5], [6, 7]],
            ins=[opart.opt()], outs=[oshared.opt()])
        nc.sync.dma_start(out=out, in_=oshared)

    nc.compile()
    return nc


def _get_program():
    if "nc" not in _CACHE:
        _CACHE["nc"] = _build_program()
    return _CACHE["nc"]


def kernel(**inputs):
    nc = _get_program()
    f32 = lambda a: np.ascontiguousarray(np.asarray(a), dtype=np.float32)
    f16 = lambda a: np.ascontiguousarray(np.asarray(a, dtype=np.float32),
                                         dtype=np.float16)
    hs = np.asarray(inputs["hidden_states"], dtype=np.float32)   # (B, L, D)
    winT = f16(np.asarray(inputs["in_proj_w"], dtype=np.float32).T)
    xpT = f16(np.asarray(inputs["x_proj_w"], dtype=np.float32).T)
    agg_w = f32(inputs["agg_w"])
    out_w = f32(inputs["out_w"])
    conv_w = f32(inputs["conv_w"])
    conv_b = f32(inputs["conv_b"])
    dt_w = f32(inputs["dt_w"])
    dt_b = f32(inputs["dt_b"])
    A_log = f32(inputs["A_log"])
    D_param = f32(inputs["D_param"])

    Mks = [f16((out_w @ agg_w[:, k * E:(k + 1) * E]).T) for k in range(K)]
    dtwTs = [f16(dt_w[k].T) for k in range(K)]
    Amats = [f32(-np.exp(A_log[k])) for k in range(K)]

    in_maps = []
    for c in range(NCORES):
        b, k = c // 2, c % 2
        in_maps.append({
            "hsT": f16(hs[b].T),
            "winT": winT,
            "xpT": xpT,
            "dtwT": dtwTs[k],
            "dtb": f32(dt_b[k][:, None]),
            "convw": f32(conv_w[k]),
            "convb": f32(conv_b[k][:, None]),
            "Amat": Amats[k],
            "Dp": f32(D_param[k][:, None]),
            "Mk": Mks[k],
        })
    _CACHE["in_maps"] = in_maps
    res = run_bass_kernel_spmd(nc, in_maps, list(range(NCORES)))
    _CACHE["last_results"] = res.results
    out = np.empty((B, L, D), np.float32)
    for b in range(B):
        out[b] = res.results[2 * b]["out"]
    return out


# revision 27
# speedup vs baseline: 1.1697x; 1.1697x over previous
"""Trainium2 Bass kernel for the 2-module Mamba-style SSM block.

Sharding: 8 cores = 4 batches x 2 modules (core c -> batch c//2, module c%2).
Each core computes one full branch for one batch; aggregate+out_proj folded
into M_k per module; pair-wise AllReduce; host picks one core per batch.

v2: channels on partitions, fp16 cube, L chunked at LC=512.
Per chunk: in_proj (PE f16) with conv + x_proj interleaved per channel
tile; dt proj + softplus as two batched ACT passes (avoids per-tile
Exp<->Ln table thrash); cube per channel tile:
  a = exp(A_n * delta) fused on ACT (per-partition scale), u = v (x) B
  via one broadcast-AP f16 tensor_tensor, the 16 state recurrences run
  as TWO 8-state tensor_tensor_scans (state boundaries reset by zeroing
  a[:, n, 0]; chunk carries folded into u[:, :, 0]), scan output lands
  in-place over u, then C-mult + log2 tree-reduce, gating (all DVE
  fp16 2x mode; big elementwise ops are kept OFF GpSimd because
  concurrent Pool ops starve DVE SBUF access), out_proj (PE, f16 yf
  stationary blocks, streamed Mk) accumulated over channel tiles.
GpSimd handles only tiny strided ops (halo moves, carry extract/fold).
One pair AllReduce at the end (chunked collectives contend with
compute globally and slow every engine ~20%).
"""
from contextlib import ExitStack

import numpy as np

import concourse.bass as bass
import concourse.tile as tile
from concourse import bacc, mybir
from concourse.bass_utils import run_bass_kernel_spmd

FP = mybir.dt.float32
F16 = mybir.dt.float16
AX = mybir.AxisListType
OP = mybir.AluOpType
AF = mybir.ActivationFunctionType

B, L, D = 4, 2048, 1024
E, N, CW, K, R = 2048, 16, 4, 2, 64
ET = E // 128           # 16 channel tiles
DT = D // 128           # 8 d_model tiles
LC = 512                # chunk length along L
NLC = L // LC           # 4 chunks
MMF = 512               # matmul moving free size
NCORES = 8

_CACHE = {}


def _build_program():
    nc = bacc.Bacc("TRN2", target_bir_lowering=False, debug=False,
                   num_devices=NCORES)

    def din(name, shape, dt=F16):
        return nc.dram_tensor(name, list(shape), dt, kind="ExternalInput").ap()

    hsT = din("hsT", (D, L))              # hidden_states[b].T, f16
    winT = din("winT", (D, 2 * E))        # in_proj_w.T, f16
    xpT = din("xpT", (E, R + 2 * N))      # x_proj_w.T, f16
    dtwT = din("dtwT", (R, E))            # dt_w[k].T, f16
    dtb = din("dtb", (E, 1), FP)
    convw = din("convw", (E, CW), FP)
    convb = din("convb", (E, 1), FP)
    Amat = din("Amat", (E, N), FP)        # -exp(A_log[k])
    Dp = din("Dp", (E, 1), FP)
    Mk = din("Mk", (E, D))                # (out_w @ agg_w[:, k*E:(k+1)*E]).T, f16
    out = nc.dram_tensor("out", [L, D], FP, kind="ExternalOutput").ap()

    zspill = nc.dram_tensor("zspill", [E, L], F16).ap()
    bcspill = nc.dram_tensor("bcspill", [2 * N, L], F16).ap()

    with tile.TileContext(nc) as tc, ExitStack() as ctx:
        const = ctx.enter_context(tc.tile_pool(name="const", bufs=1))
        dram = ctx.enter_context(tc.tile_pool(name="dram", bufs=1, space="DRAM"))
        wpool = ctx.enter_context(tc.tile_pool(name="wpool", bufs=2))
        ch_pool = ctx.enter_context(tc.tile_pool(name="chp", bufs=2))
        hs_pool = ctx.enter_context(tc.tile_pool(name="hsp", bufs=1))
        u_pool = ctx.enter_context(tc.tile_pool(name="up", bufs=2))
        a_pool = ctx.enter_context(tc.tile_pool(name="ap", bufs=2))
        dl_pool = ctx.enter_context(tc.tile_pool(name="dlp", bufs=1))
        a0_pool = ctx.enter_context(tc.tile_pool(name="a0p", bufs=1))
        t_pool = ctx.enter_context(tc.tile_pool(name="tp", bufs=1))
        zs_pool = ctx.enter_context(tc.tile_pool(name="zsp", bufs=2))
        y_pool = ctx.enter_context(tc.tile_pool(name="yp", bufs=1))
        ev_pool = ctx.enter_context(tc.tile_pool(name="ev", bufs=1))
        xd_pool = ctx.enter_context(tc.tile_pool(name="xd", bufs=1))
        mk_pool = ctx.enter_context(tc.tile_pool(name="mkp", bufs=1))
        pin = ctx.enter_context(tc.tile_pool(name="pin", bufs=2, space="PSUM"))
        pxp = ctx.enter_context(tc.tile_pool(name="pxp", bufs=2, space="PSUM"))
        pdt = ctx.enter_context(tc.tile_pool(name="pdt", bufs=2, space="PSUM"))
        pout = ctx.enter_context(tc.tile_pool(name="pout", bufs=2, space="PSUM"))

        opart = dram.tile([L, D], FP)
        oshared = dram.tile([L, D], FP)

        # ---- resident constants ----
        xpT_sb = const.tile([128, ET, R + 2 * N], F16)
        nc.sync.dma_start(out=xpT_sb,
                          in_=xpT.rearrange("(a p) c -> p a c", p=128))
        dtwT_sb = const.tile([R, ET, 128], F16)
        nc.sync.dma_start(out=dtwT_sb,
                          in_=dtwT.rearrange("p (a c) -> p a c", c=128))
        Amat_sb = const.tile([128, ET, N], FP)
        nc.sync.dma_start(out=Amat_sb,
                          in_=Amat.rearrange("(a p) n -> p a n", p=128))
        dtb_sb = const.tile([128, ET, 1], FP)
        nc.sync.dma_start(out=dtb_sb, in_=dtb.rearrange("(a p) o -> p a o", p=128))
        Dp_sb = const.tile([128, ET, 1], FP)
        nc.sync.dma_start(out=Dp_sb, in_=Dp.rearrange("(a p) o -> p a o", p=128))
        convw_sb = const.tile([128, ET, CW], FP)
        nc.sync.dma_start(out=convw_sb,
                          in_=convw.rearrange("(a p) c -> p a c", p=128))
        convb_sb = const.tile([128, ET, 1], FP)
        nc.sync.dma_start(out=convb_sb,
                          in_=convb.rearrange("(a p) o -> p a o", p=128))
        carry = const.tile([128, ET, N], FP)
        halo = const.tile([128, ET, CW - 1], F16)
        nc.vector.memset(halo, 0.0)

        for lc in range(NLC):
            lsl = slice(lc * LC, (lc + 1) * LC)
            # ---- in_proj (PE): xz[:, lsl] for all 2E channels ----
            hs_sb = hs_pool.tile([128, DT, LC], F16, tag="hs")
            for dt_ in range(DT):
                nc.sync.dma_start(out=hs_sb[:, dt_, :],
                                  in_=hsT[dt_ * 128:(dt_ + 1) * 128, lsl])
            xtp = ch_pool.tile([128, ET, CW - 1 + LC], F16, tag="xtp")
            psx = pxp.tile([R + 2 * N, LC], FP, tag="mmxp")
            for ct in range(2 * ET):
                win_ct = wpool.tile([128, DT, 128], F16, tag="win")
                nc.sync.dma_start(
                    out=win_ct,
                    in_=winT[:, ct * 128:(ct + 1) * 128].rearrange(
                        "(a p) c -> p a c", p=128))
                psums = []
                for _h in range(LC // MMF):
                    ps_in = pin.tile([128, MMF], FP, tag="mmin", name=f"psin{_h}")
                    psums.append(ps_in)
                for dt_ in range(DT):
                    for h, ps in enumerate(psums):
                        nc.tensor.matmul(ps, win_ct[:, dt_, :],
                                         hs_sb[:, dt_, h * MMF:(h + 1) * MMF],
                                         start=(dt_ == 0), stop=(dt_ == DT - 1))
                if ct < ET:
                    et = ct
                    for h, ps in enumerate(psums):
                        nc.scalar.activation(
                            out=xtp[:, et, CW - 1 + h * MMF:CW - 1 + (h + 1) * MMF],
                            in_=ps, func=AF.Copy)
                    # conv + silu for this tile, then x_proj accumulation
                    nc.gpsimd.tensor_copy(out=xtp[:, et, 0:CW - 1],
                                          in_=halo[:, et, :])
                    acc = t_pool.tile([128, LC], F16, tag="conv")
                    nc.vector.tensor_scalar(out=acc, in0=xtp[:, et, 0:LC],
                                            scalar1=convw_sb[:, et, 0:1],
                                            scalar2=None, op0=OP.mult)
                    for j in range(1, CW):
                        nc.vector.scalar_tensor_tensor(
                            out=acc, in0=xtp[:, et, j:j + LC],
                            scalar=convw_sb[:, et, j:j + 1], in1=acc,
                            op0=OP.mult, op1=OP.add)
                    nc.gpsimd.tensor_copy(out=halo[:, et, :],
                                          in_=xtp[:, et, LC:LC + CW - 1])
                    nc.scalar.activation(out=xtp[:, et, CW - 1:CW - 1 + LC],
                                         in_=acc, func=AF.Silu,
                                         bias=convb_sb[:, et, :], scale=1.0)
                    nc.tensor.matmul(psx, xpT_sb[:, et, :],
                                     xtp[:, et, CW - 1:CW - 1 + LC],
                                     start=(et == 0), stop=(et == ET - 1))
                else:
                    for h, ps in enumerate(psums):
                        zt = ev_pool.tile([128, MMF], F16, tag="zt")
                        nc.scalar.activation(out=zt, in_=ps, func=AF.Silu)
                        nc.sync.dma_start(
                            out=zspill[(ct - ET) * 128:(ct - ET + 1) * 128,
                                       lc * LC + h * MMF:lc * LC + (h + 1) * MMF],
                            in_=zt)

            xdbl = xd_pool.tile([R + 2 * N, LC], F16, tag="xdbl")
            nc.scalar.activation(out=xdbl, in_=psx, func=AF.Copy)
            nc.sync.dma_start(out=bcspill[:, lsl], in_=xdbl[R:R + 2 * N, :])
            Bbc = ch_pool.tile([128, N, LC], F16, tag="Bbc")
            Cbc = ch_pool.tile([128, N, LC], F16, tag="Cbc")
            nc.sync.dma_start(out=Bbc, in_=bass.AP(
                tensor=bcspill.tensor, offset=lc * LC,
                ap=[[0, 128], [L, N], [1, LC]]))
            nc.sync.dma_start(out=Cbc, in_=bass.AP(
                tensor=bcspill.tensor, offset=N * L + lc * LC,
                ap=[[0, 128], [L, N], [1, LC]]))

            # ---- dt proj + softplus in two batched ACT passes ----
            dlt = dl_pool.tile([128, ET, LC], F16, tag="dlt")
            for et in range(ET):
                psd = pdt.tile([128, LC], FP, tag="mmdt")
                nc.tensor.matmul(psd, dtwT_sb[:, et, :], xdbl[0:R, :],
                                 start=True, stop=True)
                nc.scalar.activation(out=dlt[:, et, :], in_=psd, func=AF.Exp,
                                     bias=dtb_sb[:, et, :], scale=1.0)
            for et in range(ET):
                nc.scalar.activation(out=dlt[:, et, :], in_=dlt[:, et, :],
                                     func=AF.Ln, bias=1.0)

            # ---- cube per channel tile ----
            for et in range(ET):
                delta = dlt[:, et, :]
                v = t_pool.tile([128, LC], F16, tag="v")
                nc.vector.tensor_tensor(out=v, in0=delta,
                                        in1=xtp[:, et, CW - 1:CW - 1 + LC],
                                        op=OP.mult)
                vb = v[:, :].rearrange("p (o t) -> p o t", o=1)
                u = u_pool.tile([128, N, LC], F16, tag="u")
                nc.vector.tensor_tensor(out=u,
                                        in0=vb.broadcast_to([128, N, LC]),
                                        in1=Bbc, op=OP.mult)
                if lc > 0:
                    # fold chunk carry into u[:, :, 0]: u0' = u0 + a0 * carry
                    d0f = a0_pool.tile([128, 1], FP, tag="d0f")
                    nc.scalar.activation(out=d0f, in_=delta[:, 0:1], func=AF.Copy)
                    a0 = a0_pool.tile([128, N], F16, tag="a0")
                    nc.scalar.activation(out=a0, in_=Amat_sb[:, et, :],
                                         func=AF.Exp, scale=d0f[:, 0:1])
                    ctmp = a0_pool.tile([128, N], F16, tag="ctmp")
                    nc.vector.tensor_tensor(out=ctmp, in0=a0,
                                            in1=carry[:, et, :], op=OP.mult)
                    ctmp3 = ctmp[:, :].rearrange("p (n o) -> p n o", o=1)
                    nc.gpsimd.tensor_tensor(out=u[:, :, 0:1], in0=u[:, :, 0:1],
                                            in1=ctmp3, op=OP.add)
                for half in range(2):
                    ah = a_pool.tile([128, N // 2, LC], F16, tag="ah",
                                     name=f"ah{half}")
                    for j in range(N // 2):
                        n = half * (N // 2) + j
                        nc.scalar.activation(out=ah[:, j, :], in_=delta,
                                             func=AF.Exp,
                                             scale=Amat_sb[:, et, n:n + 1])
                    # reset recurrence at each state boundary (incl. chunk start)
                    nc.gpsimd.memset(ah[:, :, 0:1], 0.0)
                    uh = u[:, half * (N // 2):(half + 1) * (N // 2), :]
                    nc.vector.tensor_tensor_scan(
                        out=uh.rearrange("p n t -> p (n t)"),
                        data0=ah[:, :, :].rearrange("p n t -> p (n t)"),
                        data1=uh.rearrange("p n t -> p (n t)"),
                        initial=0.0, op0=OP.mult, op1=OP.add)
                if lc < NLC - 1:
                    nc.scalar.activation(out=carry[:, et, :],
                                         in_=u[:, :, LC - 1], func=AF.Copy)
                # C-mult + tree reduce over n (in place on u)
                nc.vector.tensor_tensor(out=u[:, :, :], in0=u[:, :, :],
                                        in1=Cbc, op=OP.mult)
                nc.vector.tensor_tensor(out=u[:, 0:8, :], in0=u[:, 0:8, :],
                                        in1=u[:, 8:16, :], op=OP.add)
                nc.vector.tensor_tensor(out=u[:, 0:4, :], in0=u[:, 0:4, :],
                                        in1=u[:, 4:8, :], op=OP.add)
                nc.vector.tensor_tensor(out=u[:, 0:2, :], in0=u[:, 0:2, :],
                                        in1=u[:, 2:4, :], op=OP.add)
                y = y_pool.tile([128, LC], F16, tag="y")
                nc.vector.tensor_tensor(out=y, in0=u[:, 0, :], in1=u[:, 1, :],
                                        op=OP.add)
                zs = zs_pool.tile([128, LC], F16, tag="zs")
                nc.sync.dma_start(out=zs,
                                  in_=zspill[et * 128:(et + 1) * 128, lsl])
                t2 = t_pool.tile([128, LC], F16, tag="t2")
                nc.vector.scalar_tensor_tensor(out=t2,
                                               in0=xtp[:, et, CW - 1:CW - 1 + LC],
                                               scalar=Dp_sb[:, et, :], in1=y,
                                               op0=OP.mult, op1=OP.add)
                nc.vector.tensor_tensor(out=xtp[:, et, CW - 1:CW - 1 + LC],
                                        in0=t2, in1=zs, op=OP.mult)

            # ---- out_proj (PE): yf^T @ Mk, accumulated over et ----
            for dh in range(D // MMF):
                mk_sb = mk_pool.tile([128, ET, MMF], F16, tag="mk")
                nc.sync.dma_start(
                    out=mk_sb,
                    in_=Mk[:, dh * MMF:(dh + 1) * MMF].rearrange(
                        "(a p) c -> p a c", p=128))
                for tau in range(LC // 128):
                    po = pout.tile([128, MMF], FP, tag="mmo")
                    for et in range(ET):
                        nc.tensor.matmul(
                            po, xtp[:, et, CW - 1 + tau * 128:CW - 1 + (tau + 1) * 128],
                            mk_sb[:, et, :],
                            start=(et == 0), stop=(et == ET - 1))
                    osb = ev_pool.tile([128, MMF], FP, tag="osb")
                    nc.scalar.activation(out=osb, in_=po, func=AF.Copy)
                    nc.sync.dma_start(
                        out=opart[lc * LC + tau * 128:lc * LC + (tau + 1) * 128,
                                  dh * MMF:(dh + 1) * MMF],
                        in_=osb)

        # ---- pair AllReduce + output ----
        nc.gpsimd.collective_compute(
            "AllReduce", OP.add,
            replica_groups=[[0, 1], [2, 3], [4, 5], [6, 7]],
            ins=[opart.opt()], outs=[oshared.opt()])
        nc.sync.dma_start(out=out, in_=oshared)

    nc.compile()
    return nc


def _get_program():
    if "nc" not in _CACHE:
        _CACHE["nc"] = _build_program()
    return _CACHE["nc"]


def kernel(**inputs):
    nc = _get_program()
    f32 = lambda a: np.ascontiguousarray(np.asarray(a), dtype=np.float32)
    f16 = lambda a: np.ascontiguousarray(np.asarray(a, dtype=np.float32),
                                         dtype=np.float16)
    hs = np.asarray(inputs["hidden_states"], dtype=np.float32)   # (B, L, D)
    winT = f16(np.asarray(inputs["in_proj_w"], dtype=np.float32).T)
    xpT = f16(np.asarray(inputs["x_proj_w"], dtype=np.float32).T)
    agg_w = f32(inputs["agg_w"])
    out_w = f32(inputs["out_w"])
    conv_w = f32(inputs["conv_w"])
    conv_b = f32(inputs["conv_b"])
    dt_w = f32(inputs["dt_w"])
    dt_b = f32(inputs["dt_b"])
    A_log = f32(inputs["A_log"])
    D_param = f32(inputs["D_param"])

    Mks = [f16((out_w @ agg_w[:, k * E:(k + 1) * E]).T) for k in range(K)]
    dtwTs = [f16(dt_w[k].T) for k in range(K)]
    Amats = [f32(-np.exp(A_log[k])) for k in range(K)]

    in_maps = []
    for c in range(NCORES):
        b, k = c // 2, c % 2
        in_maps.append({
            "hsT": f16(hs[b].T),
            "winT": winT,
            "xpT": xpT,
            "dtwT": dtwTs[k],
            "dtb": f32(dt_b[k][:, None]),
            "convw": f32(conv_w[k]),
            "convb": f32(conv_b[k][:, None]),
            "Amat": Amats[k],
            "Dp": f32(D_param[k][:, None]),
            "Mk": Mks[k],
        })
    _CACHE["in_maps"] = in_maps
    res = run_bass_kernel_spmd(nc, in_maps, list(range(NCORES)))
    _CACHE["last_results"] = res.results
    out = np.empty((B, L, D), np.float32)
    for b in range(B):
        out[b] = res.results[2 * b]["out"]
    return out


# revision 28
# speedup vs baseline: 1.2174x; 1.0408x over previous
"""Trainium2 Bass kernel for the 2-module Mamba-style SSM block.

Sharding: 8 cores = 4 batches x 2 modules (core c -> batch c//2, module c%2).
Each core computes one full branch for one batch; aggregate+out_proj folded
into M_k per module; pair-wise AllReduce; host picks one core per batch.

v2: channels on partitions, fp16 cube, L chunked at LC=512.
Per chunk: in_proj (PE f16) with conv + x_proj interleaved per channel
tile; dt proj + softplus as two batched ACT passes (avoids per-tile
Exp<->Ln table thrash); cube per channel tile:
  a = exp(A_n * delta) fused on ACT (per-partition scale), u = v (x) B
  via one broadcast-AP f16 tensor_tensor, the 16 state recurrences run
  as TWO 8-state tensor_tensor_scans (state boundaries reset by zeroing
  a[:, n, 0]; chunk carries folded into u[:, :, 0]), scan output lands
  in-place over u, then C-mult + log2 tree-reduce, gating (all DVE
  fp16 2x mode; big elementwise ops are kept OFF GpSimd because
  concurrent Pool ops starve DVE SBUF access), out_proj (PE, f16 yf
  stationary blocks, streamed Mk) accumulated over channel tiles.
GpSimd handles only tiny strided ops (halo moves, carry extract/fold).
One pair AllReduce at the end (chunked collectives contend with
compute globally and slow every engine ~20%).
"""
from contextlib import ExitStack

import numpy as np

import concourse.bass as bass
import concourse.tile as tile
from concourse import bacc, mybir
from concourse.bass_utils import run_bass_kernel_spmd

FP = mybir.dt.float32
F16 = mybir.dt.float16
AX = mybir.AxisListType
OP = mybir.AluOpType
AF = mybir.ActivationFunctionType

B, L, D = 4, 2048, 1024
E, N, CW, K, R = 2048, 16, 4, 2, 64
ET = E // 128           # 16 channel tiles
DT = D // 128           # 8 d_model tiles
LC = 512                # chunk length along L
NLC = L // LC           # 4 chunks
MMF = 512               # matmul moving free size
NCORES = 8

_CACHE = {}


def _build_program():
    nc = bacc.Bacc("TRN2", target_bir_lowering=False, debug=False,
                   num_devices=NCORES)

    def din(name, shape, dt=F16):
        return nc.dram_tensor(name, list(shape), dt, kind="ExternalInput").ap()

    hsT = din("hsT", (D, L))              # hidden_states[b].T, f16
    winT = din("winT", (D, 2 * E))        # in_proj_w.T, f16
    xpT = din("xpT", (E, R + 2 * N))      # x_proj_w.T, f16
    dtwT = din("dtwT", (R, E))            # dt_w[k].T, f16
    dtb = din("dtb", (E, 1), FP)
    convw = din("convw", (E, CW), FP)
    convb = din("convb", (E, 1), FP)
    Amat = din("Amat", (E, N), FP)        # -exp(A_log[k])
    Dp = din("Dp", (E, 1), FP)
    Mk = din("Mk", (E, D))                # (out_w @ agg_w[:, k*E:(k+1)*E]).T, f16
    out = nc.dram_tensor("out", [L, D], FP, kind="ExternalOutput").ap()

    zspill = nc.dram_tensor("zspill", [E, L], F16).ap()
    bcspill = nc.dram_tensor("bcspill", [2 * N, L], F16).ap()

    with tile.TileContext(nc) as tc, ExitStack() as ctx:
        const = ctx.enter_context(tc.tile_pool(name="const", bufs=1))
        dram = ctx.enter_context(tc.tile_pool(name="dram", bufs=1, space="DRAM"))
        wpool = ctx.enter_context(tc.tile_pool(name="wpool", bufs=2))
        ch_pool = ctx.enter_context(tc.tile_pool(name="chp", bufs=2))
        hs_pool = ctx.enter_context(tc.tile_pool(name="hsp", bufs=1))
        u_pool = ctx.enter_context(tc.tile_pool(name="up", bufs=2))
        a_pool = ctx.enter_context(tc.tile_pool(name="ap", bufs=2))
        dl_pool = ctx.enter_context(tc.tile_pool(name="dlp", bufs=1))
        a0_pool = ctx.enter_context(tc.tile_pool(name="a0p", bufs=1))
        t_pool = ctx.enter_context(tc.tile_pool(name="tp", bufs=1))
        zs_pool = ctx.enter_context(tc.tile_pool(name="zsp", bufs=2))
        y_pool = ctx.enter_context(tc.tile_pool(name="yp", bufs=1))
        ev_pool = ctx.enter_context(tc.tile_pool(name="ev", bufs=1))
        xd_pool = ctx.enter_context(tc.tile_pool(name="xd", bufs=1))
        mk_pool = ctx.enter_context(tc.tile_pool(name="mkp", bufs=1))
        pin = ctx.enter_context(tc.tile_pool(name="pin", bufs=2, space="PSUM"))
        pxp = ctx.enter_context(tc.tile_pool(name="pxp", bufs=2, space="PSUM"))
        pdt = ctx.enter_context(tc.tile_pool(name="pdt", bufs=2, space="PSUM"))
        pout = ctx.enter_context(tc.tile_pool(name="pout", bufs=2, space="PSUM"))

        opart = dram.tile([L, D], FP)
        oshared = dram.tile([L, D], FP)

        # ---- resident constants ----
        xpT_sb = const.tile([128, ET, R + 2 * N], F16)
        nc.sync.dma_start(out=xpT_sb,
                          in_=xpT.rearrange("(a p) c -> p a c", p=128))
        dtwT_sb = const.tile([R, ET, 128], F16)
        nc.sync.dma_start(out=dtwT_sb,
                          in_=dtwT.rearrange("p (a c) -> p a c", c=128))
        Amat_sb = const.tile([128, ET, N], FP)
        nc.sync.dma_start(out=Amat_sb,
                          in_=Amat.rearrange("(a p) n -> p a n", p=128))
        dtb_sb = const.tile([128, ET, 1], FP)
        nc.sync.dma_start(out=dtb_sb, in_=dtb.rearrange("(a p) o -> p a o", p=128))
        Dp_sb = const.tile([128, ET, 1], FP)
        nc.sync.dma_start(out=Dp_sb, in_=Dp.rearrange("(a p) o -> p a o", p=128))
        convw_sb = const.tile([128, ET, CW], FP)
        nc.sync.dma_start(out=convw_sb,
                          in_=convw.rearrange("(a p) c -> p a c", p=128))
        convb_sb = const.tile([128, ET, 1], FP)
        nc.sync.dma_start(out=convb_sb,
                          in_=convb.rearrange("(a p) o -> p a o", p=128))
        carry = const.tile([128, ET, N], FP)
        halo = const.tile([128, ET, CW - 1], F16)
        nc.vector.memset(halo, 0.0)

        for lc in range(NLC):
            lsl = slice(lc * LC, (lc + 1) * LC)
            # ---- in_proj (PE): xz[:, lsl] for all 2E channels ----
            hs_sb = hs_pool.tile([128, DT, LC], F16, tag="hs")
            for dt_ in range(DT):
                nc.sync.dma_start(out=hs_sb[:, dt_, :],
                                  in_=hsT[dt_ * 128:(dt_ + 1) * 128, lsl])
            xtp = ch_pool.tile([128, ET, CW - 1 + LC], F16, tag="xtp")
            psx = pxp.tile([R + 2 * N, LC], FP, tag="mmxp")
            for ct in range(2 * ET):
                win_ct = wpool.tile([128, DT, 128], F16, tag="win")
                nc.sync.dma_start(
                    out=win_ct,
                    in_=winT[:, ct * 128:(ct + 1) * 128].rearrange(
                        "(a p) c -> p a c", p=128))
                psums = []
                for _h in range(LC // MMF):
                    ps_in = pin.tile([128, MMF], FP, tag="mmin", name=f"psin{_h}")
                    psums.append(ps_in)
                for dt_ in range(DT):
                    for h, ps in enumerate(psums):
                        nc.tensor.matmul(ps, win_ct[:, dt_, :],
                                         hs_sb[:, dt_, h * MMF:(h + 1) * MMF],
                                         start=(dt_ == 0), stop=(dt_ == DT - 1))
                if ct < ET:
                    et = ct
                    for h, ps in enumerate(psums):
                        nc.scalar.activation(
                            out=xtp[:, et, CW - 1 + h * MMF:CW - 1 + (h + 1) * MMF],
                            in_=ps, func=AF.Copy)
                    # conv + silu for this tile, then x_proj accumulation
                    nc.gpsimd.tensor_copy(out=xtp[:, et, 0:CW - 1],
                                          in_=halo[:, et, :])
                    acc = t_pool.tile([128, LC], F16, tag="conv")
                    nc.vector.tensor_scalar(out=acc, in0=xtp[:, et, 0:LC],
                                            scalar1=convw_sb[:, et, 0:1],
                                            scalar2=None, op0=OP.mult)
                    for j in range(1, CW):
                        nc.vector.scalar_tensor_tensor(
                            out=acc, in0=xtp[:, et, j:j + LC],
                            scalar=convw_sb[:, et, j:j + 1], in1=acc,
                            op0=OP.mult, op1=OP.add)
                    nc.gpsimd.tensor_copy(out=halo[:, et, :],
                                          in_=xtp[:, et, LC:LC + CW - 1])
                    nc.scalar.activation(out=xtp[:, et, CW - 1:CW - 1 + LC],
                                         in_=acc, func=AF.Silu,
                                         bias=convb_sb[:, et, :], scale=1.0)
                    nc.tensor.matmul(psx, xpT_sb[:, et, :],
                                     xtp[:, et, CW - 1:CW - 1 + LC],
                                     start=(et == 0), stop=(et == ET - 1))
                else:
                    for h, ps in enumerate(psums):
                        zt = ev_pool.tile([128, MMF], F16, tag="zt")
                        nc.scalar.activation(out=zt, in_=ps, func=AF.Silu)
                        nc.sync.dma_start(
                            out=zspill[(ct - ET) * 128:(ct - ET + 1) * 128,
                                       lc * LC + h * MMF:lc * LC + (h + 1) * MMF],
                            in_=zt)

            xdbl = xd_pool.tile([R + 2 * N, LC], F16, tag="xdbl")
            nc.scalar.activation(out=xdbl, in_=psx, func=AF.Copy)
            nc.sync.dma_start(out=bcspill[:, lsl], in_=xdbl[R:R + 2 * N, :])
            Bbc = ch_pool.tile([128, N, LC], F16, tag="Bbc")
            Cbc = ch_pool.tile([128, N, LC], F16, tag="Cbc")
            nc.sync.dma_start(out=Bbc, in_=bass.AP(
                tensor=bcspill.tensor, offset=lc * LC,
                ap=[[0, 128], [L, N], [1, LC]]))
            nc.sync.dma_start(out=Cbc, in_=bass.AP(
                tensor=bcspill.tensor, offset=N * L + lc * LC,
                ap=[[0, 128], [L, N], [1, LC]]))

            # ---- dt proj + softplus in two batched ACT passes ----
            dlt = dl_pool.tile([128, ET, LC], F16, tag="dlt")
            for et in range(ET):
                psd = pdt.tile([128, LC], FP, tag="mmdt")
                nc.tensor.matmul(psd, dtwT_sb[:, et, :], xdbl[0:R, :],
                                 start=True, stop=True)
                nc.scalar.activation(out=dlt[:, et, :], in_=psd, func=AF.Exp,
                                     bias=dtb_sb[:, et, :], scale=1.0)
            for et in range(ET):
                nc.scalar.activation(out=dlt[:, et, :], in_=dlt[:, et, :],
                                     func=AF.Ln, bias=1.0)

            # ---- cube per channel tile ----
            for et in range(ET):
                delta = dlt[:, et, :]
                v = t_pool.tile([128, LC], F16, tag="v")
                nc.vector.tensor_tensor(out=v, in0=delta,
                                        in1=xtp[:, et, CW - 1:CW - 1 + LC],
                                        op=OP.mult)
                vb = v[:, :].rearrange("p (o t) -> p o t", o=1)
                u = u_pool.tile([128, N, LC], F16, tag="u")
                nc.vector.tensor_tensor(out=u,
                                        in0=vb.broadcast_to([128, N, LC]),
                                        in1=Bbc, op=OP.mult)
                if lc > 0:
                    # fold chunk carry into u[:, :, 0]: u0' = u0 + a0 * carry
                    d0f = a0_pool.tile([128, 1], FP, tag="d0f")
                    nc.scalar.activation(out=d0f, in_=delta[:, 0:1], func=AF.Copy)
                    a0 = a0_pool.tile([128, N], F16, tag="a0")
                    nc.scalar.activation(out=a0, in_=Amat_sb[:, et, :],
                                         func=AF.Exp, scale=d0f[:, 0:1])
                    ctmp = a0_pool.tile([128, N], F16, tag="ctmp")
                    nc.vector.tensor_tensor(out=ctmp, in0=a0,
                                            in1=carry[:, et, :], op=OP.mult)
                    ctmp3 = ctmp[:, :].rearrange("p (n o) -> p n o", o=1)
                    nc.gpsimd.tensor_tensor(out=u[:, :, 0:1], in0=u[:, :, 0:1],
                                            in1=ctmp3, op=OP.add)
                for half in range(2):
                    ah = a_pool.tile([128, N // 2, LC], F16, tag="ah",
                                     name=f"ah{half}")
                    for j in range(N // 2):
                        n = half * (N // 2) + j
                        nc.scalar.activation(out=ah[:, j, :], in_=delta,
                                             func=AF.Exp,
                                             scale=Amat_sb[:, et, n:n + 1])
                    # reset recurrence at each state boundary (incl. chunk start)
                    nc.gpsimd.memset(ah[:, :, 0:1], 0.0)
                    uh = u[:, half * (N // 2):(half + 1) * (N // 2), :]
                    nc.vector.tensor_tensor_scan(
                        out=uh.rearrange("p n t -> p (n t)"),
                        data0=ah[:, :, :].rearrange("p n t -> p (n t)"),
                        data1=uh.rearrange("p n t -> p (n t)"),
                        initial=0.0, op0=OP.mult, op1=OP.add)
                if lc < NLC - 1:
                    nc.scalar.activation(out=carry[:, et, :],
                                         in_=u[:, :, LC - 1], func=AF.Copy)
                # C-mult + tree reduce over n (in place on u)
                nc.vector.tensor_tensor(out=u[:, :, :], in0=u[:, :, :],
                                        in1=Cbc, op=OP.mult)
                nc.vector.tensor_tensor(out=u[:, 0:8, :], in0=u[:, 0:8, :],
                                        in1=u[:, 8:16, :], op=OP.add)
                nc.vector.tensor_tensor(out=u[:, 0:4, :], in0=u[:, 0:4, :],
                                        in1=u[:, 4:8, :], op=OP.add)
                nc.vector.tensor_tensor(out=u[:, 0:2, :], in0=u[:, 0:2, :],
                                        in1=u[:, 2:4, :], op=OP.add)
                y = y_pool.tile([128, LC], F16, tag="y")
                nc.vector.tensor_tensor(out=y, in0=u[:, 0, :], in1=u[:, 1, :],
                                        op=OP.add)
                zs = zs_pool.tile([128, LC], F16, tag="zs")
                nc.sync.dma_start(out=zs,
                                  in_=zspill[et * 128:(et + 1) * 128, lsl])
                t2 = t_pool.tile([128, LC], F16, tag="t2")
                nc.vector.scalar_tensor_tensor(out=t2,
                                               in0=xtp[:, et, CW - 1:CW - 1 + LC],
                                               scalar=Dp_sb[:, et, :], in1=y,
                                               op0=OP.mult, op1=OP.add)
                nc.vector.tensor_tensor(out=xtp[:, et, CW - 1:CW - 1 + LC],
                                        in0=t2, in1=zs, op=OP.mult)

            # ---- out_proj (PE): yf^T @ Mk, accumulated over et ----
            for dh in range(D // MMF):
                mk_sb = mk_pool.tile([128, ET, MMF], F16, tag="mk")
                nc.sync.dma_start(
                    out=mk_sb,
                    in_=Mk[:, dh * MMF:(dh + 1) * MMF].rearrange(
                        "(a p) c -> p a c", p=128))
                for tau in range(LC // 128):
                    po = pout.tile([128, MMF], FP, tag="mmo")
                    for et in range(ET):
                        nc.tensor.matmul(
                            po, xtp[:, et, CW - 1 + tau * 128:CW - 1 + (tau + 1) * 128],
                            mk_sb[:, et, :],
                            start=(et == 0), stop=(et == ET - 1))
                    osb = ev_pool.tile([128, MMF], FP, tag="osb")
                    nc.scalar.activation(out=osb, in_=po, func=AF.Copy)
                    nc.sync.dma_start(
                        out=opart[lc * LC + tau * 128:lc * LC + (tau + 1) * 128,
                                  dh * MMF:(dh + 1) * MMF],
                        in_=osb)
            if lc == 1:
                nc.gpsimd.collective_compute(
                    "AllReduce", OP.add,
                    replica_groups=[[0, 1], [2, 3], [4, 5], [6, 7]],
                    ins=[opart[0:2 * LC, :].opt()],
                    outs=[oshared[0:2 * LC, :].opt()])
                nc.sync.dma_start(out=out[0:2 * LC, :], in_=oshared[0:2 * LC, :])

        # ---- pair AllReduce + output (second half; first half overlapped) ----
        nc.gpsimd.collective_compute(
            "AllReduce", OP.add,
            replica_groups=[[0, 1], [2, 3], [4, 5], [6, 7]],
            ins=[opart[2 * LC:L, :].opt()], outs=[oshared[2 * LC:L, :].opt()])
        nc.sync.dma_start(out=out[2 * LC:L, :], in_=oshared[2 * LC:L, :])

    nc.compile()
    return nc


def _get_program():
    if "nc" not in _CACHE:
        _CACHE["nc"] = _build_program()
    return _CACHE["nc"]


def kernel(**inputs):
    nc = _get_program()
    f32 = lambda a: np.ascontiguousarray(np.asarray(a), dtype=np.float32)
    f16 = lambda a: np.ascontiguousarray(np.asarray(a, dtype=np.float32),
                                         dtype=np.float16)
    hs = np.asarray(inputs["hidden_states"], dtype=np.float32)   # (B, L, D)
    winT = f16(np.asarray(inputs["in_proj_w"], dtype=np.float32).T)
    xpT = f16(np.asarray(inputs["x_proj_w"], dtype=np.float32).T)
    agg_w = f32(inputs["agg_w"])
    out_w = f32(inputs["out_w"])
    conv_w = f32(inputs["conv_w"])
    conv_b = f32(inputs["conv_b"])
    dt_w = f32(inputs["dt_w"])
    dt_b = f32(inputs["dt_b"])
    A_log = f32(inputs["A_log"])
    D_param = f32(inputs["D_param"])

    Mks = [f16((out_w @ agg_w[:, k * E:(k + 1) * E]).T) for k in range(K)]
    dtwTs = [f16(dt_w[k].T) for k in range(K)]
    Amats = [f32(-np.exp(A_log[k])) for k in range(K)]

    in_maps = []
    for c in range(NCORES):
        b, k = c // 2, c % 2
        in_maps.append({
            "hsT": f16(hs[b].T),
            "winT": winT,
            "xpT": xpT,
            "dtwT": dtwTs[k],
            "dtb": f32(dt_b[k][:, None]),
            "convw": f32(conv_w[k]),
            "convb": f32(conv_b[k][:, None]),
            "Amat": Amats[k],
            "Dp": f32(D_param[k][:, None]),
            "Mk": Mks[k],
        })
    _CACHE["in_maps"] = in_maps
    res = run_bass_kernel_spmd(nc, in_maps, list(range(NCORES)))
    _CACHE["last_results"] = res.results
    out = np.empty((B, L, D), np.float32)
    for b in range(B):
        out[b] = res.results[2 * b]["out"]
    return out


# revision 30
# speedup vs baseline: 1.2248x; 1.0061x over previous
"""Trainium2 Bass kernel for the 2-module Mamba-style SSM block.

Sharding: 8 cores = 4 batches x 2 modules (core c -> batch c//2, module c%2).
Each core computes one full branch for one batch; aggregate+out_proj folded
into M_k per module; pair-wise AllReduce; host picks one core per batch.

v2: channels on partitions, fp16 cube, L chunked at LC=512.
Per chunk: in_proj (PE f16) with conv + x_proj interleaved per channel
tile; dt proj + softplus as two batched ACT passes (avoids per-tile
Exp<->Ln table thrash); cube per channel tile:
  a = exp(A_n * delta) fused on ACT (per-partition scale), u = v (x) B
  via one broadcast-AP f16 tensor_tensor, the 16 state recurrences run
  as TWO 8-state tensor_tensor_scans (state boundaries reset by zeroing
  a[:, n, 0]; chunk carries folded into u[:, :, 0]), scan output lands
  in-place over u, then C-mult + log2 tree-reduce, gating (all DVE
  fp16 2x mode; big elementwise ops are kept OFF GpSimd because
  concurrent Pool ops starve DVE SBUF access), out_proj (PE, f16 yf
  stationary blocks, streamed Mk) accumulated over channel tiles.
GpSimd handles only tiny strided ops (halo moves, carry extract/fold).
One pair AllReduce at the end (chunked collectives contend with
compute globally and slow every engine ~20%).
"""
from contextlib import ExitStack

import numpy as np

import concourse.bass as bass
import concourse.tile as tile
from concourse import bacc, mybir
from concourse.bass_utils import run_bass_kernel_spmd

FP = mybir.dt.float32
F16 = mybir.dt.float16
AX = mybir.AxisListType
OP = mybir.AluOpType
AF = mybir.ActivationFunctionType

B, L, D = 4, 2048, 1024
E, N, CW, K, R = 2048, 16, 4, 2, 64
ET = E // 128           # 16 channel tiles
DT = D // 128           # 8 d_model tiles
LC = 512                # chunk length along L
NLC = L // LC           # 4 chunks
MMF = 512               # matmul moving free size
NCORES = 8

_CACHE = {}


def _build_program():
    nc = bacc.Bacc("TRN2", target_bir_lowering=False, debug=False,
                   num_devices=NCORES)

    def din(name, shape, dt=F16):
        return nc.dram_tensor(name, list(shape), dt, kind="ExternalInput").ap()

    hsT = din("hsT", (D, L))              # hidden_states[b].T, f16
    winT = din("winT", (D, 2 * E))        # in_proj_w.T, f16
    xpT = din("xpT", (E, R + 2 * N))      # x_proj_w.T, f16
    dtwT = din("dtwT", (R, E))            # dt_w[k].T, f16
    dtb = din("dtb", (E, 1), FP)
    convw = din("convw", (E, CW), FP)
    convb = din("convb", (E, 1), FP)
    Amat = din("Amat", (E, N), FP)        # -exp(A_log[k])
    Dp = din("Dp", (E, 1), FP)
    Mk = din("Mk", (E, D))                # (out_w @ agg_w[:, k*E:(k+1)*E]).T, f16
    out = nc.dram_tensor("out", [L, D], FP, kind="ExternalOutput").ap()

    zspill = nc.dram_tensor("zspill", [E, L], F16).ap()
    bcspill = nc.dram_tensor("bcspill", [2 * N, L], F16).ap()

    with tile.TileContext(nc) as tc, ExitStack() as ctx:
        const = ctx.enter_context(tc.tile_pool(name="const", bufs=1))
        dram = ctx.enter_context(tc.tile_pool(name="dram", bufs=1, space="DRAM"))
        wpool = ctx.enter_context(tc.tile_pool(name="wpool", bufs=2))
        ch_pool = ctx.enter_context(tc.tile_pool(name="chp", bufs=2))
        hs_pool = ctx.enter_context(tc.tile_pool(name="hsp", bufs=1))
        u_pool = ctx.enter_context(tc.tile_pool(name="up", bufs=2))
        a_pool = ctx.enter_context(tc.tile_pool(name="ap", bufs=2))
        dl_pool = ctx.enter_context(tc.tile_pool(name="dlp", bufs=1))
        a0_pool = ctx.enter_context(tc.tile_pool(name="a0p", bufs=1))
        t_pool = ctx.enter_context(tc.tile_pool(name="tp", bufs=1))
        zs_pool = ctx.enter_context(tc.tile_pool(name="zsp", bufs=2))
        y_pool = ctx.enter_context(tc.tile_pool(name="yp", bufs=1))
        ev_pool = ctx.enter_context(tc.tile_pool(name="ev", bufs=1))
        xd_pool = ctx.enter_context(tc.tile_pool(name="xd", bufs=1))
        mk_pool = ctx.enter_context(tc.tile_pool(name="mkp", bufs=1))
        pin = ctx.enter_context(tc.tile_pool(name="pin", bufs=2, space="PSUM"))
        pxp = ctx.enter_context(tc.tile_pool(name="pxp", bufs=2, space="PSUM"))
        pdt = ctx.enter_context(tc.tile_pool(name="pdt", bufs=2, space="PSUM"))
        pout = ctx.enter_context(tc.tile_pool(name="pout", bufs=2, space="PSUM"))

        opart = dram.tile([L, D], FP)
        oshared = dram.tile([L, D], FP)

        # ---- resident constants ----
        xpT_sb = const.tile([128, ET, R + 2 * N], F16)
        nc.sync.dma_start(out=xpT_sb,
                          in_=xpT.rearrange("(a p) c -> p a c", p=128))
        dtwT_sb = const.tile([R, ET, 128], F16)
        nc.sync.dma_start(out=dtwT_sb,
                          in_=dtwT.rearrange("p (a c) -> p a c", c=128))
        Amat_sb = const.tile([128, ET, N], FP)
        nc.sync.dma_start(out=Amat_sb,
                          in_=Amat.rearrange("(a p) n -> p a n", p=128))
        dtb_sb = const.tile([128, ET, 1], FP)
        nc.sync.dma_start(out=dtb_sb, in_=dtb.rearrange("(a p) o -> p a o", p=128))
        Dp_sb = const.tile([128, ET, 1], FP)
        nc.sync.dma_start(out=Dp_sb, in_=Dp.rearrange("(a p) o -> p a o", p=128))
        convw_sb = const.tile([128, ET, CW], FP)
        nc.sync.dma_start(out=convw_sb,
                          in_=convw.rearrange("(a p) c -> p a c", p=128))
        convb_sb = const.tile([128, ET, 1], FP)
        nc.sync.dma_start(out=convb_sb,
                          in_=convb.rearrange("(a p) o -> p a o", p=128))
        carry = const.tile([128, ET, N], FP)
        halo = const.tile([128, ET, CW - 1], F16)
        nc.vector.memset(halo, 0.0)

        for lc in range(NLC):
            lsl = slice(lc * LC, (lc + 1) * LC)
            # ---- in_proj (PE): xz[:, lsl] for all 2E channels ----
            hs_sb = hs_pool.tile([128, DT, LC], F16, tag="hs")
            for dt_ in range(DT):
                nc.sync.dma_start(out=hs_sb[:, dt_, :],
                                  in_=hsT[dt_ * 128:(dt_ + 1) * 128, lsl])
            xtp = ch_pool.tile([128, ET, CW - 1 + LC], F16, tag="xtp")
            psx = pxp.tile([R + 2 * N, LC], FP, tag="mmxp")
            for ct in range(2 * ET):
                win_ct = wpool.tile([128, DT, 128], F16, tag="win")
                nc.sync.dma_start(
                    out=win_ct,
                    in_=winT[:, ct * 128:(ct + 1) * 128].rearrange(
                        "(a p) c -> p a c", p=128))
                psums = []
                for _h in range(LC // MMF):
                    ps_in = pin.tile([128, MMF], FP, tag="mmin", name=f"psin{_h}")
                    psums.append(ps_in)
                for dt_ in range(DT):
                    for h, ps in enumerate(psums):
                        nc.tensor.matmul(ps, win_ct[:, dt_, :],
                                         hs_sb[:, dt_, h * MMF:(h + 1) * MMF],
                                         start=(dt_ == 0), stop=(dt_ == DT - 1))
                if ct < ET:
                    et = ct
                    for h, ps in enumerate(psums):
                        nc.scalar.activation(
                            out=xtp[:, et, CW - 1 + h * MMF:CW - 1 + (h + 1) * MMF],
                            in_=ps, func=AF.Copy)
                    # conv + silu for this tile, then x_proj accumulation
                    nc.gpsimd.tensor_copy(out=xtp[:, et, 0:CW - 1],
                                          in_=halo[:, et, :])
                    acc = t_pool.tile([128, LC], F16, tag="conv")
                    nc.vector.tensor_scalar(out=acc, in0=xtp[:, et, 0:LC],
                                            scalar1=convw_sb[:, et, 0:1],
                                            scalar2=None, op0=OP.mult)
                    for j in range(1, CW):
                        nc.vector.scalar_tensor_tensor(
                            out=acc, in0=xtp[:, et, j:j + LC],
                            scalar=convw_sb[:, et, j:j + 1], in1=acc,
                            op0=OP.mult, op1=OP.add)
                    nc.gpsimd.tensor_copy(out=halo[:, et, :],
                                          in_=xtp[:, et, LC:LC + CW - 1])
                    nc.scalar.activation(out=xtp[:, et, CW - 1:CW - 1 + LC],
                                         in_=acc, func=AF.Silu,
                                         bias=convb_sb[:, et, :], scale=1.0)
                    nc.tensor.matmul(psx, xpT_sb[:, et, :],
                                     xtp[:, et, CW - 1:CW - 1 + LC],
                                     start=(et == 0), stop=(et == ET - 1))
                else:
                    for h, ps in enumerate(psums):
                        zt = ev_pool.tile([128, MMF], F16, tag="zt")
                        nc.scalar.activation(out=zt, in_=ps, func=AF.Silu)
                        nc.sync.dma_start(
                            out=zspill[(ct - ET) * 128:(ct - ET + 1) * 128,
                                       lc * LC + h * MMF:lc * LC + (h + 1) * MMF],
                            in_=zt)

            xdbl = xd_pool.tile([R + 2 * N, LC], F16, tag="xdbl")
            nc.scalar.activation(out=xdbl, in_=psx, func=AF.Copy)
            nc.sync.dma_start(out=bcspill[:, lsl], in_=xdbl[R:R + 2 * N, :])
            Bbc = ch_pool.tile([128, N, LC], F16, tag="Bbc")
            Cbc = ch_pool.tile([128, N, LC], F16, tag="Cbc")
            nc.sync.dma_start(out=Bbc, in_=bass.AP(
                tensor=bcspill.tensor, offset=lc * LC,
                ap=[[0, 128], [L, N], [1, LC]]))
            nc.sync.dma_start(out=Cbc, in_=bass.AP(
                tensor=bcspill.tensor, offset=N * L + lc * LC,
                ap=[[0, 128], [L, N], [1, LC]]))

            # ---- dt proj + softplus in two batched ACT passes ----
            dlt = dl_pool.tile([128, ET, LC], F16, tag="dlt")
            for et in range(ET):
                psd = pdt.tile([128, LC], FP, tag="mmdt")
                nc.tensor.matmul(psd, dtwT_sb[:, et, :], xdbl[0:R, :],
                                 start=True, stop=True)
                nc.scalar.activation(out=dlt[:, et, :], in_=psd, func=AF.Exp,
                                     bias=dtb_sb[:, et, :], scale=1.0)
            for et in range(ET):
                nc.scalar.activation(out=dlt[:, et, :], in_=dlt[:, et, :],
                                     func=AF.Ln, bias=1.0)

            # ---- cube per channel tile ----
            for et in range(ET):
                delta = dlt[:, et, :]
                v = t_pool.tile([128, LC], F16, tag="v")
                nc.vector.tensor_tensor(out=v, in0=delta,
                                        in1=xtp[:, et, CW - 1:CW - 1 + LC],
                                        op=OP.mult)
                vb = v[:, :].rearrange("p (o t) -> p o t", o=1)
                u = u_pool.tile([128, N, LC], F16, tag="u")
                nc.vector.tensor_tensor(out=u,
                                        in0=vb.broadcast_to([128, N, LC]),
                                        in1=Bbc, op=OP.mult)
                if lc > 0:
                    # fold chunk carry into u[:, :, 0]: u0' = u0 + a0 * carry
                    d0f = a0_pool.tile([128, 1], FP, tag="d0f")
                    nc.scalar.activation(out=d0f, in_=delta[:, 0:1], func=AF.Copy)
                    a0 = a0_pool.tile([128, N], F16, tag="a0")
                    nc.scalar.activation(out=a0, in_=Amat_sb[:, et, :],
                                         func=AF.Exp, scale=d0f[:, 0:1])
                    ctmp = a0_pool.tile([128, N], F16, tag="ctmp")
                    nc.vector.tensor_tensor(out=ctmp, in0=a0,
                                            in1=carry[:, et, :], op=OP.mult)
                    ctmp3 = ctmp[:, :].rearrange("p (n o) -> p n o", o=1)
                    nc.gpsimd.tensor_tensor(out=u[:, :, 0:1], in0=u[:, :, 0:1],
                                            in1=ctmp3, op=OP.add)
                for half in range(2):
                    ah = a_pool.tile([128, N // 2, LC], F16, tag="ah",
                                     name=f"ah{half}")
                    for j in range(N // 2):
                        n = half * (N // 2) + j
                        nc.scalar.activation(out=ah[:, j, :], in_=delta,
                                             func=AF.Exp,
                                             scale=Amat_sb[:, et, n:n + 1])
                    # reset recurrence at each state boundary (incl. chunk start)
                    nc.gpsimd.memset(ah[:, :, 0:1], 0.0)
                    uh = u[:, half * (N // 2):(half + 1) * (N // 2), :]
                    nc.vector.tensor_tensor_scan(
                        out=uh.rearrange("p n t -> p (n t)"),
                        data0=ah[:, :, :].rearrange("p n t -> p (n t)"),
                        data1=uh.rearrange("p n t -> p (n t)"),
                        initial=0.0, op0=OP.mult, op1=OP.add)
                if lc < NLC - 1:
                    nc.scalar.activation(out=carry[:, et, :],
                                         in_=u[:, :, LC - 1], func=AF.Copy)
                # C-mult + tree reduce over n (in place on u)
                nc.vector.tensor_tensor(out=u[:, :, :], in0=u[:, :, :],
                                        in1=Cbc, op=OP.mult)
                nc.vector.tensor_tensor(out=u[:, 0:8, :], in0=u[:, 0:8, :],
                                        in1=u[:, 8:16, :], op=OP.add)
                nc.vector.tensor_tensor(out=u[:, 0:4, :], in0=u[:, 0:4, :],
                                        in1=u[:, 4:8, :], op=OP.add)
                nc.vector.tensor_tensor(out=u[:, 0:2, :], in0=u[:, 0:2, :],
                                        in1=u[:, 2:4, :], op=OP.add)
                y = y_pool.tile([128, LC], F16, tag="y")
                nc.vector.tensor_tensor(out=y, in0=u[:, 0, :], in1=u[:, 1, :],
                                        op=OP.add)
                zs = zs_pool.tile([128, LC], F16, tag="zs")
                nc.sync.dma_start(out=zs,
                                  in_=zspill[et * 128:(et + 1) * 128, lsl])
                t2 = t_pool.tile([128, LC], F16, tag="t2")
                nc.vector.scalar_tensor_tensor(out=t2,
                                               in0=xtp[:, et, CW - 1:CW - 1 + LC],
                                               scalar=Dp_sb[:, et, :], in1=y,
                                               op0=OP.mult, op1=OP.add)
                nc.vector.tensor_tensor(out=xtp[:, et, CW - 1:CW - 1 + LC],
                                        in0=t2, in1=zs, op=OP.mult)

            # ---- out_proj (PE): yf^T @ Mk, accumulated over et ----
            for dh in range(D // MMF):
                mk_sb = mk_pool.tile([128, ET, MMF], F16, tag="mk")
                nc.sync.dma_start(
                    out=mk_sb,
                    in_=Mk[:, dh * MMF:(dh + 1) * MMF].rearrange(
                        "(a p) c -> p a c", p=128))
                for tau in range(LC // 128):
                    po = pout.tile([128, MMF], FP, tag="mmo")
                    for et in range(ET):
                        nc.tensor.matmul(
                            po, xtp[:, et, CW - 1 + tau * 128:CW - 1 + (tau + 1) * 128],
                            mk_sb[:, et, :],
                            start=(et == 0), stop=(et == ET - 1))
                    osb = ev_pool.tile([128, MMF], FP, tag="osb")
                    nc.scalar.activation(out=osb, in_=po, func=AF.Copy)
                    nc.sync.dma_start(
                        out=opart[lc * LC + tau * 128:lc * LC + (tau + 1) * 128,
                                  dh * MMF:(dh + 1) * MMF],
                        in_=osb)
            if lc == 1:
                nc.gpsimd.collective_compute(
                    "AllReduce", OP.add,
                    replica_groups=[[0, 1], [2, 3], [4, 5], [6, 7]],
                    ins=[opart[0:2 * LC, :].opt()],
                    outs=[oshared[0:2 * LC, :].opt()])
                nc.sync.dma_start(out=out[0:2 * LC, :], in_=oshared[0:2 * LC, :])
            if lc == 2:
                nc.gpsimd.collective_compute(
                    "AllReduce", OP.add,
                    replica_groups=[[0, 1], [2, 3], [4, 5], [6, 7]],
                    ins=[opart[2 * LC:3 * LC, :].opt()],
                    outs=[oshared[2 * LC:3 * LC, :].opt()])
                nc.sync.dma_start(out=out[2 * LC:3 * LC, :],
                                  in_=oshared[2 * LC:3 * LC, :])

        # ---- pair AllReduce + output (second half; first half overlapped) ----
        nc.gpsimd.collective_compute(
            "AllReduce", OP.add,
            replica_groups=[[0, 1], [2, 3], [4, 5], [6, 7]],
            ins=[opart[3 * LC:L, :].opt()], outs=[oshared[3 * LC:L, :].opt()])
        nc.sync.dma_start(out=out[3 * LC:L, :], in_=oshared[3 * LC:L, :])

    nc.compile()
    return nc


def _get_program():
    if "nc" not in _CACHE:
        _CACHE["nc"] = _build_program()
    return _CACHE["nc"]


def kernel(**inputs):
    nc = _get_program()
    f32 = lambda a: np.ascontiguousarray(np.asarray(a), dtype=np.float32)
    f16 = lambda a: np.ascontiguousarray(np.asarray(a, dtype=np.float32),
                                         dtype=np.float16)
    hs = np.asarray(inputs["hidden_states"], dtype=np.float32)   # (B, L, D)
    winT = f16(np.asarray(inputs["in_proj_w"], dtype=np.float32).T)
    xpT = f16(np.asarray(inputs["x_proj_w"], dtype=np.float32).T)
    agg_w = f32(inputs["agg_w"])
    out_w = f32(inputs["out_w"])
    conv_w = f32(inputs["conv_w"])
    conv_b = f32(inputs["conv_b"])
    dt_w = f32(inputs["dt_w"])
    dt_b = f32(inputs["dt_b"])
    A_log = f32(inputs["A_log"])
    D_param = f32(inputs["D_param"])

    Mks = [f16((out_w @ agg_w[:, k * E:(k + 1) * E]).T) for k in range(K)]
    dtwTs = [f16(dt_w[k].T) for k in range(K)]
    Amats = [f32(-np.exp(A_log[k])) for k in range(K)]

    in_maps = []
    for c in range(NCORES):
        b, k = c // 2, c % 2
        in_maps.append({
            "hsT": f16(hs[b].T),
            "winT": winT,
            "xpT": xpT,
            "dtwT": dtwTs[k],
            "dtb": f32(dt_b[k][:, None]),
            "convw": f32(conv_w[k]),
            "convb": f32(conv_b[k][:, None]),
            "Amat": Amats[k],
            "Dp": f32(D_param[k][:, None]),
            "Mk": Mks[k],
        })
    _CACHE["in_maps"] = in_maps
    res = run_bass_kernel_spmd(nc, in_maps, list(range(NCORES)))
    _CACHE["last_results"] = res.results
    out = np.empty((B, L, D), np.float32)
    for b in range(B):
        out[b] = res.results[2 * b]["out"]
    return out
